# revision 4
# baseline (speedup 1.0000x reference)
"""B-spline evaluation kernel for Trainium2 (8 NeuronCores, data-parallel).

Math: uniform cubic B-spline, 64 basis fns, knots linspace(0,1,68).
For s = 67*x, cell = round(s - 0.5), u = s - cell:
    y = A0[cell] + A1[cell]*u + A2[cell]*u^2 + A3[cell]*u^3
with per-cell polynomial coefficients A_q derived from coefs on host.
Device decode via sign-mask prefix sums:
    acc_q = C_q + sum_{j=1..66} m_j * (dA_qj / 2),   m_j = sign(cell-j+.5)
The 264 mask-MACs are spread over three engines to beat the DVE-only
baseline: ACT generates the 66 sign masks plus premultiplied term
tensors (Copy(m*d/2+0)) for the units GPSIMD absorbs with tensor_tensor
adds; DVE handles the remaining scalar_tensor_tensor MACs and Horner.
Tables are runtime inputs (SBUF per-partition scalar columns).
"""
import numpy as np

N_POINTS = 1_000_000
N_CORES = 8
PER_CORE = N_POINTS // N_CORES  # 125000
P, F = 128, 977  # 125056 >= PER_CORE
NCELL = 67
NJ = NCELL - 1  # 66 mask steps j=1..66

# unit split: DVE keeps q=0,1 fully and q=2 up to J2SPLIT-1;
# GPSIMD (fed by ACT terms) takes q=3 fully and q=2 from J2SPLIT.
J2SPLIT = 40

# table column layout
COL_MBIAS = 0                       # 66 mask biases (0.5 - j), j=1..66
COL_INIT_S0 = COL_MBIAS + NJ        # 4 init scales  d_q1/2
COL_INIT_S1 = COL_INIT_S0 + 4       # 4 init biases  C_q=(A_q[0]+A_q[66])/2
_DVE_UNITS = [(q, j) for q in (0, 1) for j in range(2, NCELL)] + \
             [(2, j) for j in range(2, J2SPLIT)]
_GP_UNITS = [(3, j) for j in range(2, NCELL)] + \
            [(2, j) for j in range(J2SPLIT, NCELL)]
COL_DVE = COL_INIT_S1 + 4           # len(_DVE_UNITS) cols of d_qj/2
COL_GP = COL_DVE + len(_DVE_UNITS)  # len(_GP_UNITS) cols of d_qj/2
TAB_COLS = COL_GP + len(_GP_UNITS)

_cache = {}


def _build_nc():
    import concourse.tile as tile
    from concourse import bacc, mybir

    fp32 = mybir.dt.float32
    nc = bacc.Bacc("TRN2", target_bir_lowering=False, debug=False,
                   num_devices=N_CORES)
    x = nc.dram_tensor("x", [P, F], fp32, kind="ExternalInput").ap()
    tab = nc.dram_tensor("tab", [P, TAB_COLS], fp32, kind="ExternalInput").ap()
    y = nc.dram_tensor("y", [P, F], fp32, kind="ExternalOutput").ap()

    Alu = mybir.AluOpType
    Act = mybir.ActivationFunctionType

    def tcol(t, c):
        return t[:, c:c + 1]

    with tile.TileContext(nc) as tc:
        with tc.tile_pool(name="main", bufs=1) as pool:
            xt = pool.tile([P, F], fp32, tag="xt")
            nc.sync.dma_start(xt[:], x)
            tt = pool.tile([P, TAB_COLS], fp32, tag="tab")
            nc.sync.dma_start(tt[:], tab)

            # cellf on ACT: t1 = 2*round(67x-.5) + 2^24 via fp32 grid trick
            t1 = pool.tile([P, F], fp32, tag="t1")
            nc.scalar.activation(t1[:], xt[:], Act.Copy, bias=16777215.0,
                                 scale=134.0)
            cellf = pool.tile([P, F], fp32, tag="cellf")
            nc.scalar.activation(cellf[:], t1[:], Act.Copy, bias=-8388608.0,
                                 scale=0.5)
            # u = 67*x - cellf  (DVE stt)
            u = pool.tile([P, F], fp32, tag="u")
            nc.vector.scalar_tensor_tensor(u[:], xt[:], 67.0, cellf[:],
                                           Alu.mult, Alu.subtract)

            # unit → table-column maps
            dve_col = {qj: COL_DVE + i for i, qj in enumerate(_DVE_UNITS)}
            gp_col = {qj: COL_GP + i for i, qj in enumerate(_GP_UNITS)}
            gp_by_j = {}
            for (q, j) in _GP_UNITS:
                gp_by_j.setdefault(j, []).append(q)
            dve_by_j = {}
            for (q, j) in _DVE_UNITS:
                dve_by_j.setdefault(j, []).append(q)

            acc = [pool.tile([P, F], fp32, tag=f"acc{q}", name=f"acc_{q}")
                   for q in range(4)]
            acc2g = pool.tile([P, F], fp32, tag="acc2g")
            terms = [pool.tile([P, F], fp32, tag=f"term{i}", name=f"term_{i}")
                     for i in range(6)]
            masks = {}
            first2 = True
            nterm = 0
            # interleaved emission: mask_j then all its consumers, so the
            # rotating mask buffers never wedge the in-order ACT queue.
            for j in range(1, NCELL):
                m = pool.tile([P, F], fp32, tag=f"mask{(j - 1) % 6}",
                              name=f"mask_{j}")
                nc.scalar.activation(m[:], cellf[:], Act.Sign,
                                     bias=tcol(tt, COL_MBIAS + j - 1),
                                     scale=1.0)
                masks[j] = m
                if j == 1:
                    # acc init on DVE (tensor_scalar folds the j=1 MAC):
                    # acc_q = m_1 * (d_q1/2) + C_q
                    for q in range(4):
                        nc.vector.tensor_scalar(acc[q][:], m[:],
                                                tcol(tt, COL_INIT_S0 + q),
                                                tcol(tt, COL_INIT_S1 + q),
                                                Alu.mult, Alu.add)
                    continue
                for q in dve_by_j.get(j, ()):
                    nc.vector.scalar_tensor_tensor(
                        acc[q][:], m[:], tcol(tt, dve_col[(q, j)]),
                        acc[q][:], Alu.mult, Alu.add)
                for q in gp_by_j.get(j, ()):
                    col = tcol(tt, gp_col[(q, j)])
                    if q == 2 and first2:
                        # ACT writes the first q=2 term directly into acc2g
                        nc.scalar.activation(acc2g[:], m[:], Act.Copy,
                                             bias=0.0, scale=col)
                        first2 = False
                        continue
                    t = terms[nterm % 6]
                    nterm += 1
                    nc.scalar.activation(t[:], m[:], Act.Copy, bias=0.0,
                                         scale=col)
                    tgt = acc[3] if q == 3 else acc2g
                    nc.gpsimd.tensor_tensor(tgt[:], tgt[:], t[:], Alu.add)

            # merge q=2 partials on GP
            nc.gpsimd.tensor_tensor(acc[2][:], acc[2][:], acc2g[:], Alu.add)

            # Horner: first two steps on GP, rest on DVE
            h = pool.tile([P, F], fp32, tag="h")
            nc.gpsimd.tensor_tensor(h[:], acc[3][:], u[:], Alu.mult)
            nc.gpsimd.tensor_tensor(h[:], h[:], acc[2][:], Alu.add)
            nc.vector.tensor_tensor(h[:], h[:], u[:], Alu.mult)
            nc.vector.tensor_tensor(h[:], h[:], acc[1][:], Alu.add)
            nc.vector.tensor_tensor(h[:], h[:], u[:], Alu.mult)
            nc.vector.tensor_tensor(h[:], h[:], acc[0][:], Alu.add)
            nc.sync.dma_start(y, h[:])
    nc.compile()
    return nc


def _make_tables(coefs):
    c = np.zeros(70, dtype=np.float64)
    c[3:67] = np.asarray(coefs, dtype=np.float64)
    A = np.zeros((NCELL, 4), dtype=np.float64)
    for k in range(NCELL):
        c0, c1, c2, c3 = c[k], c[k + 1], c[k + 2], c[k + 3]
        A[k, 0] = (c0 + 4.0 * c1 + c2) / 6.0
        A[k, 1] = (-3.0 * c0 + 3.0 * c2) / 6.0
        A[k, 2] = (3.0 * c0 - 6.0 * c1 + 3.0 * c2) / 6.0
        A[k, 3] = (-c0 + 3.0 * c1 - 3.0 * c2 + c3) / 6.0
    dA = np.zeros((4, NCELL), dtype=np.float64)  # dA[q, j] = A_q[j]-A_q[j-1]
    for j in range(1, NCELL):
        dA[:, j] = A[j] - A[j - 1]

    tab = np.zeros(TAB_COLS, dtype=np.float64)
    for j in range(1, NCELL):
        tab[COL_MBIAS + j - 1] = 0.5 - j
    for q in range(4):
        tab[COL_INIT_S0 + q] = dA[q, 1] / 2.0
        tab[COL_INIT_S1 + q] = (A[0, q] + A[NCELL - 1, q]) / 2.0
    for idx, (q, j) in enumerate(_DVE_UNITS):
        tab[COL_DVE + idx] = dA[q, j] / 2.0
    for idx, (q, j) in enumerate(_GP_UNITS):
        tab[COL_GP + idx] = dA[q, j] / 2.0
    return tab.astype(np.float32)


def kernel(x, knot_vector, coefs):
    from concourse.bass_utils import run_bass_kernel_spmd

    if "nc" not in _cache:
        _cache["nc"] = _build_nc()
    nc = _cache["nc"]

    x = np.asarray(x, dtype=np.float32)
    tab = _make_tables(coefs)
    tab_tile = np.broadcast_to(tab, (P, TAB_COLS)).copy()

    in_maps = []
    for core in range(N_CORES):
        shard = x[core * PER_CORE:(core + 1) * PER_CORE]
        pad = np.full(P * F, 0.5, dtype=np.float32)
        pad[:PER_CORE] = shard
        in_maps.append({"x": pad.reshape(P, F), "tab": tab_tile})

    res = run_bass_kernel_spmd(nc, in_maps, core_ids=list(range(N_CORES)))
    out = np.empty(N_POINTS, dtype=np.float32)
    for core in range(N_CORES):
        out[core * PER_CORE:(core + 1) * PER_CORE] = \
            res.results[core]["y"].reshape(-1)[:PER_CORE]
    return out


# revision 5
# speedup vs baseline: 2.2046x; 2.2046x over previous
"""B-spline evaluation kernel for Trainium2 (8 NeuronCores, data-parallel).

Math: uniform cubic B-spline, 64 basis fns, knots linspace(0,1,68).
For s = 67*x, cell = round(s - 0.5), u = s - cell:
    y = A0[c] + A1[c]*u + A2[c]*u^2 + A3[c]*u^3,  c = cell
Decode via step-mask sums  acc_q = A_q[0] + sum_j step_j * dA_qj  spread
over three engines, with the Tensor engine doing the accumulation as
float32r identity matmuls into PSUM (PSUM accumulate is free and PE has
its own SBUF ports, so no DVE/ACT contention):
  route A js: DVE tensor_scalar (2x perf mode) makes terms
              (cellf>=j-.5)*dA straight from cellf;
  route B js: ACT makes a sign mask then Copy-scaled terms (+-dA/2);
  route C js: ACT mask + DVE scalar_tensor_tensor into a side
              accumulator (no PE traffic).
Half-d/2 conventions are absorbed into per-q constant terms that also
ride the PSUM accumulation. Tables are runtime inputs.
"""
import numpy as np

N_POINTS = 1_000_000
N_CORES = 8
PER_CORE = N_POINTS // N_CORES  # 125000
P, F = 128, 984  # 125952 >= PER_CORE; F even, F/2 % 4 == 0 for f32r matmul
HF = F // 2
NCELL = 67

# j-route split (tunable): route A = DVE-fused, B = ACT terms, C = DVE stt
_ALL_J = list(range(1, NCELL))
A_N, B_N, C_N = 32, 26, 8
assert A_N + B_N + C_N == 66
# interleave for smooth pipelining: pattern by index
_route = {}
_a = _b = _c = 0
for i, j in enumerate(_ALL_J):
    frac_a = (_a + 1) / A_N if A_N else 9e9
    frac_b = (_b + 1) / B_N if B_N else 9e9
    frac_c = (_c + 1) / C_N if C_N else 9e9
    m = min(frac_a, frac_b, frac_c)
    if m == frac_a:
        _route[j] = "A"; _a += 1
    elif m == frac_b:
        _route[j] = "B"; _b += 1
    else:
        _route[j] = "C"; _c += 1

J_A = [j for j in _ALL_J if _route[j] == "A"]
J_B = [j for j in _ALL_J if _route[j] == "B"]
J_C = [j for j in _ALL_J if _route[j] == "C"]

# table columns
_cols = {}
_nc_ = [0]


def _col(name):
    _cols[name] = _nc_[0]
    _nc_[0] += 1
    return _cols[name]


for j in J_A:
    for q in range(4):
        _col(f"A_{q}_{j}")          # dA_qj (full)
for j in J_B + J_C:
    _col(f"mb_{j}")                 # mask bias 0.5 - j
for j in J_B:
    for q in range(4):
        _col(f"B_{q}_{j}")          # dA_qj / 2
for j in J_C:
    for q in range(4):
        _col(f"C_{q}_{j}")          # dA_qj / 2
for q in range(4):
    _col(f"K_{q}")                  # A_q[0] + sum_{B,C} dA_qj/2
TAB_COLS = _nc_[0]

_cache = {}


def _build_nc():
    import concourse.tile as tile
    from concourse import bacc, mybir

    fp32 = mybir.dt.float32
    f32r = mybir.dt.float32r
    nc = bacc.Bacc("TRN2", target_bir_lowering=False, debug=False,
                   num_devices=N_CORES)
    x = nc.dram_tensor("x", [P, F], fp32, kind="ExternalInput").ap()
    tab = nc.dram_tensor("tab", [P, TAB_COLS], fp32, kind="ExternalInput").ap()
    idd = nc.dram_tensor("idd", [P, P], f32r, kind="ExternalInput").ap()
    y = nc.dram_tensor("y", [P, F], fp32, kind="ExternalOutput").ap()

    Alu = mybir.AluOpType
    Act = mybir.ActivationFunctionType

    def tcol(t, name):
        c = _cols[name]
        return t[:, c:c + 1]

    # per-(q,half)-bank matmul bookkeeping for start/stop flags
    n_terms = 4 * (len(J_A) + len(J_B)) + 4  # incl. 4 constant terms
    mm_count = [0] * 8  # (q, half) -> matmuls emitted

    with tile.TileContext(nc) as tc:
        with tc.tile_pool(name="main", bufs=1) as pool, \
             tc.tile_pool(name="psum", bufs=1, space="PSUM") as psum:
            xt = pool.tile([P, F], fp32, tag="xt")
            nc.sync.dma_start(xt[:], x)
            tt = pool.tile([P, TAB_COLS], fp32, tag="tab")
            nc.sync.dma_start(tt[:], tab)
            ident = pool.tile([P, P], f32r, tag="ident")
            nc.sync.dma_start(ident[:], idd)

            accp = [psum.tile([P, HF], fp32, tag=f"accp{q}_{h}",
                              name=f"accp_{q}_{h}")
                    for q in range(4) for h in range(2)]

            def consume(term, q):
                for h in range(2):
                    bank = 2 * q + h
                    i = mm_count[bank]
                    mm_count[bank] += 1
                    nc.tensor.matmul(accp[bank][:], ident[:],
                                     term[:, h * HF:(h + 1) * HF],
                                     start=(i == 0),
                                     stop=(i == n_terms - 1))

            # cellf on ACT: 2*round(67x-.5)+2^24 grid trick, then unbias
            t1 = pool.tile([P, F], fp32, tag="t1")
            nc.scalar.activation(t1[:], xt[:], Act.Copy, bias=16777215.0,
                                 scale=134.0)
            cellf = pool.tile([P, F], fp32, tag="cellf")
            nc.scalar.activation(cellf[:], t1[:], Act.Copy, bias=-8388608.0,
                                 scale=0.5)
            # u = 67*x - cellf  (DVE)
            u = pool.tile([P, F], fp32, tag="u")
            nc.vector.scalar_tensor_tensor(u[:], xt[:], 67.0, cellf[:],
                                           Alu.mult, Alu.subtract)

            terms = [pool.tile([P, F], f32r, tag=f"term{i}", name=f"term_{i}")
                     for i in range(8)]
            tno = [0]

            def term_tile():
                t = terms[tno[0] % 8]
                tno[0] += 1
                return t

            # constant terms: K_q into PSUM via DVE ts + matmuls
            for q in range(4):
                t = term_tile()
                nc.vector.tensor_scalar(t[:], cellf[:], 0.0,
                                        tcol(tt, f"K_{q}"),
                                        Alu.mult, Alu.add)
                consume(t, q)

            masks = {}
            accd = [None] * 4
            for j in _ALL_J:
                r = _route[j]
                if r == "A":
                    # DVE-fused terms: (cellf >= j-0.5) * dA_qj
                    for q in range(4):
                        t = term_tile()
                        nc.vector.tensor_scalar(t[:], cellf[:], j - 0.5,
                                                tcol(tt, f"A_{q}_{j}"),
                                                Alu.is_ge, Alu.mult)
                        consume(t, q)
                    continue
                m = pool.tile([P, F], fp32, tag=f"mask{j % 4}",
                              name=f"mask_{j}")
                nc.scalar.activation(m[:], cellf[:], Act.Sign,
                                     bias=tcol(tt, f"mb_{j}"), scale=1.0)
                masks[j] = m
                if r == "B":
                    for q in range(4):
                        t = term_tile()
                        nc.scalar.activation(t[:], m[:], Act.Copy, bias=0.0,
                                             scale=tcol(tt, f"B_{q}_{j}"))
                        consume(t, q)
                else:  # C: DVE stt side accumulators
                    for q in range(4):
                        if accd[q] is None:
                            accd[q] = pool.tile([P, F], fp32,
                                                tag=f"accd{q}",
                                                name=f"accd_{q}")
                            nc.vector.tensor_scalar(
                                accd[q][:], m[:], tcol(tt, f"C_{q}_{j}"),
                                None, Alu.mult)
                        else:
                            nc.vector.scalar_tensor_tensor(
                                accd[q][:], m[:], tcol(tt, f"C_{q}_{j}"),
                                accd[q][:], Alu.mult, Alu.add)

            # Horner on DVE; psum halves addressed per half to stay in-bank
            h_ = pool.tile([P, F], fp32, tag="h")

            def add_acc(dst, q, first):
                for hh in range(2):
                    sl = slice(hh * HF, (hh + 1) * HF)
                    if first:
                        nc.vector.tensor_tensor(dst[:, sl], accp[2 * q + hh][:],
                                                accd[q][:, sl], Alu.add)
                    else:
                        nc.vector.tensor_tensor(dst[:, sl], dst[:, sl],
                                                accp[2 * q + hh][:], Alu.add)

            add_acc(h_, 3, True)
            for q in (2, 1, 0):
                nc.vector.tensor_tensor(h_[:], h_[:], u[:], Alu.mult)
                add_acc(h_, q, False)
                nc.vector.tensor_tensor(h_[:], h_[:], accd[q][:], Alu.add)
            nc.sync.dma_start(y, h_[:])
    nc.compile()
    return nc


def _spline_A(coefs):
    c = np.zeros(70, dtype=np.float64)
    c[3:67] = np.asarray(coefs, dtype=np.float64)
    A = np.zeros((NCELL, 4), dtype=np.float64)
    for k in range(NCELL):
        c0, c1, c2, c3 = c[k], c[k + 1], c[k + 2], c[k + 3]
        A[k, 0] = (c0 + 4.0 * c1 + c2) / 6.0
        A[k, 1] = (-3.0 * c0 + 3.0 * c2) / 6.0
        A[k, 2] = (3.0 * c0 - 6.0 * c1 + 3.0 * c2) / 6.0
        A[k, 3] = (-c0 + 3.0 * c1 - 3.0 * c2 + c3) / 6.0
    return A


def _make_tables(coefs):
    A = _spline_A(coefs)
    dA = np.zeros((4, NCELL), dtype=np.float64)
    for j in range(1, NCELL):
        dA[:, j] = A[j] - A[j - 1]
    tab = np.zeros(TAB_COLS, dtype=np.float64)
    for j in J_A:
        for q in range(4):
            tab[_cols[f"A_{q}_{j}"]] = dA[q, j]
    for j in J_B + J_C:
        tab[_cols[f"mb_{j}"]] = 0.5 - j
    for j in J_B:
        for q in range(4):
            tab[_cols[f"B_{q}_{j}"]] = dA[q, j] / 2.0
    for j in J_C:
        for q in range(4):
            tab[_cols[f"C_{q}_{j}"]] = dA[q, j] / 2.0
    for q in range(4):
        tab[_cols[f"K_{q}"]] = A[0, q] + sum(
            dA[q, j] / 2.0 for j in J_B + J_C)
    return tab.astype(np.float32)


def _make_in_maps(inputs):
    x = np.asarray(inputs["x"], dtype=np.float32)
    tab = _make_tables(inputs["coefs"])
    tab_tile = np.broadcast_to(tab, (P, TAB_COLS)).copy()
    eye = np.eye(P, dtype=np.float32)
    in_maps = []
    for core in range(N_CORES):
        shard = x[core * PER_CORE:(core + 1) * PER_CORE]
        pad = np.full(P * F, 0.5, dtype=np.float32)
        pad[:PER_CORE] = shard
        in_maps.append({"x": pad.reshape(P, F), "tab": tab_tile, "idd": eye})
    return in_maps


def kernel(x, knot_vector, coefs):
    from concourse.bass_utils import run_bass_kernel_spmd

    if "nc" not in _cache:
        _cache["nc"] = _build_nc()
    nc = _cache["nc"]

    in_maps = _make_in_maps({"x": x, "coefs": coefs})
    res = run_bass_kernel_spmd(nc, in_maps, core_ids=list(range(N_CORES)))
    out = np.empty(N_POINTS, dtype=np.float32)
    for core in range(N_CORES):
        out[core * PER_CORE:(core + 1) * PER_CORE] = \
            res.results[core]["y"].reshape(-1)[:PER_CORE]
    return out
